# revision 1
# baseline (speedup 1.0000x reference)
"""Trainium2 Bass kernel for nn_Convolution_1176821039998.

Equivariant (e3nn-style) 3D convolution, kernel 5x5x5, 64->64 channels, on a
[1,64,56,56,56] fp32 volume, plus a per-irrep self-connection on the cropped
volume.  Strategy:

Host side (tiny, fp32):
  - Build the dense conv kernel K[o,i,dz,dy,dx] from the TP weight exactly as
    the reference does, and fold the self-connection into the center tap.
  - Perfectly balanced z-shard across 8 cores with NO redundant compute:
    core c computes 6 "main" output planes 6c..6c+5 (planes 0..47) plus a
    13-line y-block of one of the remaining 2 plane-pairs (planes 48..51,
    block chosen by core index).  The partial block's input sub-volume is
    packed by the host into a small side tensor at FIXED local coordinates,
    so all cores run the identical SPMD program (3.25 plane-pairs each).
  - The kernel halo (4 planes) is handled by overlapping shards; no
    device-to-device exchange.

Device side (pair-packed matmuls using the full 128x128 PE array):
  - K = 128: 64 input channels x 2 adjacent input z-planes (dual z-shifted
    SBUF copies, built with two DMAs from one DRAM slab).
  - M = 128: 64 output channels x 2 adjacent OUTPUT z-planes.  A stream
    reading input plane z+s serves output plane z (taps dz=s low copy,
    s+1 high) and z+1 (dz=s-1 low, s high); streams s in {0,2,4} cover all
    5 z-taps of both planes -> 75 accumulating matmuls per plane-PAIR
    (25 (dy,dx) x 3), 2x fewer streamed columns than one-plane-at-a-time.
  - PSUM tile [128, L*52] fp32 per line-chunk; evict via VectorE, DMA out.

Numerics: inputs/weights fp16 (products exact in fp32, PSUM accumulates
fp32); measured max rel err vs fp32 reference ~4.5e-4.
"""

import os
import numpy as np

import concourse.bass as bass
import concourse.mybir as mybir
import concourse.tile as tile
from concourse import bacc
from concourse.bass_utils import run_bass_kernel_spmd

# ---------------------------------------------------------------- constants
SIZE = 5
MUL = 16
CROP = SIZE // 2
PW0 = np.float32((1.0 / 32.0) ** 0.5)
PW1 = np.float32((3.0 / 32.0) ** 0.5)
INV_SQRT3 = np.float32(3.0 ** -0.5)

N_CORES = 8
S = 56                                 # input spatial size
SO = 52                                # output spatial size
# main shard: 3 pairs = 6 output planes at z0 = 6c (covers 0..47)
N_PAIRS = 3
D_OUT = 6
D_DRAM = 10                            # main DRAM slab planes (6c..6c+9)
D_SB = 9                               # planes per SBUF copy (lo 0..8, hi 1..9)
# partial shard: planes 48..51 split into 8 (pair, 13-line block) quarters
P2_BASE = 48
P2_LINES = 13
P2_IN_LINES = P2_LINES + 4             # 17
P2_DRAM = 6                            # partial DRAM planes (P..P+5)
P2_SB = 5                              # partial SBUF planes per copy
CHUNKS = [(0, 9), (9, 9), (18, 9), (27, 9), (36, 9), (45, 7)]  # (y0, lines)
CHUNKS2 = [(0, 7), (7, 6)]             # partial 13-line block
# matmul group order: s-major so a chunk's first 25 matmuls only need input
# planes z..z+1 (and the first weight slices), letting compute start while
# later planes/weights are still in flight
S_STREAMS = (0, 2, 4)
GROUPS = [(dy, dx, s) for s in S_STREAMS for dy in range(5) for dx in range(5)]
NG = len(GROUPS)  # 75


def _core_assign(c):
    """(main z0, partial pair base, partial y0) for core c."""
    return 6 * c, P2_BASE + 2 * (c // 4), P2_LINES * (c % 4)


# ------------------------------------------------------- host-side weights
def _lattice_consts():
    r = np.linspace(-1.0, 1.0, SIZE, dtype=np.float32)
    lat = np.stack(np.meshgrid(r, r, r, indexing="ij"), axis=-1)
    d = np.linalg.norm(lat.astype(np.float64), axis=-1).astype(np.float32)
    values = np.linspace(0.0, 1.0, SIZE, dtype=np.float32)
    step = values[1] - values[0]
    diff = (d[..., None] - values) / step

    def sus(t):
        return np.where(t > 0, np.exp(-1.0 / np.where(t > 0, t, 1.0)), 0.0).astype(
            np.float32
        )

    emb = np.float32(1.14136) * np.float32(np.e ** 2) * sus(diff + 1.0) * sus(1.0 - diff)
    n = lat / np.maximum(d, 1e-12)[..., None]
    sh0 = np.ones_like(d)
    sh1 = np.float32(3.0 ** 0.5) * n
    return emb.astype(np.float32), sh0, sh1.astype(np.float32)


def _make_kernel(weight):
    """[5,1024] -> conv kernel [out=64, in=64, 5,5,5] fp32 (mirrors reference)."""
    emb, sh0, sh1 = _lattice_consts()
    w = emb @ weight
    Ssp = w.shape[:3]
    blk = MUL * MUL
    w1, w2, w3, w4 = [
        w[..., i * blk : (i + 1) * blk].reshape(*Ssp, MUL, MUL) for i in range(4)
    ]
    k_ss = PW0 * w1 * sh0[..., None, None]
    k_sv = PW1 * INV_SQRT3 * np.einsum("...uw,...k->...uwk", w2, sh1)
    k_vs = PW0 * INV_SQRT3 * np.einsum("...uw,...i->...uiw", w4, sh1)
    eye3 = np.eye(3, dtype=w.dtype)
    k_vv = (
        PW1
        * INV_SQRT3
        * (w3 * sh0[..., None, None])[..., :, None, :, None]
        * eye3[None, None, None, None, :, None, :]
    )
    top = np.concatenate([k_ss, k_sv.reshape(*Ssp, MUL, 3 * MUL)], axis=-1)
    bot = np.concatenate(
        [k_vs.reshape(*Ssp, 3 * MUL, MUL), k_vv.reshape(*Ssp, 3 * MUL, 3 * MUL)],
        axis=-1,
    )
    kernel = np.concatenate([top, bot], axis=-2)  # [5,5,5,in,out]
    return np.ascontiguousarray(np.transpose(kernel, (4, 3, 0, 1, 2)))


def _fold_self_connection(K, w_sc0, w_sc1):
    """Add the cropped e3nn Linear self-connection into the center tap."""
    inv = np.float32(1.0 / MUL ** 0.5)
    sc = np.zeros((64, 64), np.float32)
    sc[:MUL, :MUL] = w_sc0.T * inv  # sc[out w, in u] = w_sc0[u, w]
    for wo in range(MUL):
        for u in range(MUL):
            for k in range(3):
                sc[MUL + 3 * wo + k, MUL + 3 * u + k] += w_sc1[u, wo] * inv
    K = K.copy()
    K[:, :, CROP, CROP, CROP] += sc
    return K


def _pack_weights(K, dtype=np.float16):
    """[64,64,5,5,5] -> lhsT tiles [128, NG, 128] in GROUPS order.

    lhsT rows: 64 in-channels x {low copy (plane z+s), high copy (z+s+1)}.
    lhsT cols: 64 out-channels x {out plane z, out plane z+1}.
    Block (row half r, col half m) holds tap dz = s + r - m (zero if outside
    0..4)."""
    wk = np.zeros((128, NG, 128), np.float32)
    for g, (dy, dx, s) in enumerate(GROUPS):
        for r in range(2):
            for m in range(2):
                dz = s + r - m
                if 0 <= dz < 5:
                    wk[64 * r : 64 * r + 64, g, 64 * m : 64 * m + 64] = K[
                        :, :, dz, dy, dx
                    ].T
    return np.ascontiguousarray(wk.astype(dtype))


def _pack_x(x, dtype=np.float16):
    """x [1,64,56,56,56] -> per-core (main slab [64,10,56,56],
    partial slab [64,6,17,56])."""
    slabs = []
    for c in range(N_CORES):
        z0, p2, y2 = _core_assign(c)
        xa = np.ascontiguousarray(x[0, :, z0 : z0 + D_DRAM].astype(dtype))
        xp = np.ascontiguousarray(
            x[0, :, p2 : p2 + P2_DRAM, y2 : y2 + P2_IN_LINES].astype(dtype)
        )
        slabs.append((xa, xp))
    return slabs


# ------------------------------------------------------- device program
def build_nc(n_pairs=N_PAIRS, partial=True, repeat=1):
    fp16 = mybir.dt.float16
    fp32 = mybir.dt.float32
    nc = bacc.Bacc("TRN2", target_bir_lowering=False, debug=False,
                   num_devices=N_CORES)
    x_d = nc.dram_tensor("x", [64, D_DRAM, S, S], fp16, kind="ExternalInput").ap()
    x2_d = nc.dram_tensor("x2", [64, P2_DRAM, P2_IN_LINES, S], fp16,
                          kind="ExternalInput").ap()
    w_d = nc.dram_tensor("w", [128, NG, 128], fp16, kind="ExternalInput").ap()
    # outputs are plane-major so one DMA can write both planes of a pair:
    # SBUF partitions (z c) = plane-half * 64 + channel
    o_d = nc.dram_tensor("out", [2 * n_pairs, 64, SO, SO], fp32,
                         kind="ExternalOutput").ap()
    o2_d = nc.dram_tensor("out2", [2, 64, P2_LINES, SO], fp32,
                          kind="ExternalOutput").ap()

    with tile.TileContext(nc) as tc:
        with (
            tc.tile_pool(name="const", bufs=1) as cpool,
            tc.tile_pool(name="outp", bufs=3) as opool,
            tc.tile_pool(name="psum", bufs=8, space="PSUM") as ppool,
        ):
            xt = cpool.tile([128, D_SB, S, S], fp16)
            xt2 = cpool.tile([128, P2_SB, P2_IN_LINES, S], fp16)
            wt = cpool.tile([128, NG, 128], fp16)
            # DMA order = first-use order: tiny partial slab, then weights in
            # slices (matmul g only gates on its slice), then the main slab.
            # Dual z-shifted SBUF copies are built with two DMAs per plane
            # from the single DRAM slab: partitions 0..63 plane j <- plane j,
            # partitions 64..127 plane j <- plane j+1.
            # partial slab in two slices per half: planes 0..1 first (gates
            # the s=0 matmuls), then 2..4
            nc.sync.dma_start(xt2[:64, :2], x2_d[:, :2])
            nc.sync.dma_start(xt2[64:, :2], x2_d[:, 1:3])
            nc.sync.dma_start(xt2[:64, 2:], x2_d[:, 2:P2_SB])
            nc.sync.dma_start(xt2[64:, 2:], x2_d[:, 3 : P2_SB + 1])
            # interleave weight slices (3 groups each) with main-slab plane
            # DMAs so neither starves during the partial block's compute
            wops = [
                lambda i=i: nc.sync.dma_start(wt[:, 3 * i : 3 * (i + 1)],
                                              w_d[:, 3 * i : 3 * (i + 1)])
                for i in range(25)
            ]
            xops = []
            for j in range(D_SB):
                xops.append(lambda j=j: nc.sync.dma_start(xt[:64, j], x_d[:, j]))
                xops.append(
                    lambda j=j: nc.sync.dma_start(xt[64:, j], x_d[:, j + 1])
                )
            while wops or xops:
                if wops:
                    wops.pop(0)()
                if xops:
                    xops.pop(0)()

            def do_chunk(src, z, ys, L, dst, zo):
                ps = ppool.tile([128, 9, SO], fp32)
                for g, (dy, dx, s) in enumerate(GROUPS):
                    rhs = src[:, z + s, ys + dy : ys + dy + L, dx : dx + SO]
                    nc.tensor.matmul(ps[:, :L, :], wt[:, g], rhs,
                                     start=(g == 0), stop=(g == NG - 1))
                ot = opool.tile([128, 9, SO], fp32)
                nc.vector.tensor_copy(ot[:, :L], ps[:, :L])
                dst2 = dst[zo : zo + 2, :, ys : ys + L, :].rearrange(
                    "z c l w -> (z c) l w"
                )
                nc.sync.dma_start(dst2, ot[:, :L])

            for _ in range(repeat):
                # partial first: its input lands quickly, hiding the main
                # slab's DMA behind ~21us of compute
                if partial:
                    for ys, L in CHUNKS2:
                        do_chunk(xt2, 0, ys, L, o2_d, 0)
                for p in range(n_pairs):
                    for ys, L in CHUNKS:
                        do_chunk(xt, 2 * p, ys, L, o_d, 2 * p)
    nc.compile()
    return nc


# ------------------------------------------------------------ entry point
LAST_RESULTS = None
LAST_NC = None
LAST_INMAPS = None


def kernel(x, weight, w_sc0, w_sc1):
    global LAST_RESULTS, LAST_NC, LAST_INMAPS
    x = np.asarray(x, dtype=np.float32)
    K = _fold_self_connection(
        _make_kernel(np.asarray(weight, dtype=np.float32)),
        np.asarray(w_sc0, dtype=np.float32),
        np.asarray(w_sc1, dtype=np.float32),
    )
    wk = _pack_weights(K)
    slabs = _pack_x(x)

    repeat = int(os.environ.get("KERNEL_REPEAT", "1"))
    nc = build_nc(repeat=repeat)
    in_maps = [{"x": slabs[c][0], "x2": slabs[c][1], "w": wk}
               for c in range(N_CORES)]
    res = run_bass_kernel_spmd(nc, in_maps, core_ids=list(range(N_CORES)))
    LAST_RESULTS, LAST_NC, LAST_INMAPS = res, nc, in_maps

    full = np.zeros((1, 64, SO, SO, SO), np.float32)
    for c in range(N_CORES):
        z0, p2, y2 = _core_assign(c)
        # device outputs are plane-major [z, c, l, w]
        full[0, :, z0 : z0 + D_OUT] = res.results[c]["out"].transpose(1, 0, 2, 3)
        full[0, :, p2 : p2 + 2, y2 : y2 + P2_LINES, :] = res.results[c][
            "out2"
        ].transpose(1, 0, 2, 3)
    return full



# revision 6
# speedup vs baseline: 4.1397x; 4.1397x over previous
"""Trainium2 Bass kernel for nn_Convolution_1176821039998.

Equivariant (e3nn-style) 3D convolution, kernel 5x5x5, 64->64 channels, on a
[1,64,56,56,56] fp32 volume, plus a per-irrep self-connection folded into the
center tap.  v2: fp8e4m3 + DoubleRow tensor-engine path.

Structure exploited (host-measured on the actual weights):
  - 44 of 125 taps are structurally ZERO (smooth_finite radial basis vanishes
    for lattice offsets with |r| >= 1.25): all 4 corner (dy,dx) columns die,
    and edge columns keep only dz in {1,2,3}.  81 live taps remain.
  - The center tap (2,2,2) carries ~92.3% of the squared kernel mass (the
    folded self-connection dominates), so it alone gets an error-compensated
    2-pass treatment; the other 80 taps use single fp8 products.  Host-
    simulated max rel err ~1e-2 vs the 2e-2 gate.

Device-side decomposition (per output plane-pair, M = 64ch x 2 planes):
  - "quad" = one DoubleRow islot of a matmul: partitions = 64ch x 2 adjacent
    input planes (dual z-shifted SBUF copies), covering (r,m) taps
    dz = a + r - m for quad base a.
  - DoubleRow packs 2 quads per matmul via the rhs interleave dim (stride =
    plane jump or small even spatial delta; odd byte strides hang the HW),
    at 0.5 cycles per output column -> 4x MAC rate vs fp16.
  - Per chunk: 1 hl-matmul (center tap, islots = (x_hi, x_lo) for full-
    precision x) + 26 xh-matmuls (52 quads: 51 dirty W8 quads + 1 center
    W_lo correction quad) = 27 matmuls vs 75 fp16 matmuls in v1.

Sharding: identical to v1 - perfectly balanced z-shard, core c computes
output planes 6c..6c+5 plus a 13-line y-block of one plane-pair in 48..51.
"""

import numpy as np
import ml_dtypes

import concourse.bass as bass
import concourse.mybir as mybir
import concourse.tile as tile
from concourse import bacc
from concourse.ap import AP
from concourse.bass_utils import run_bass_kernel_spmd

# ---------------------------------------------------------------- constants
SIZE = 5
MUL = 16
CROP = SIZE // 2
PW0 = np.float32((1.0 / 32.0) ** 0.5)
PW1 = np.float32((3.0 / 32.0) ** 0.5)
INV_SQRT3 = np.float32(3.0 ** -0.5)

N_CORES = 8
S = 56                                 # input spatial size
SO = 52                                # output spatial size
N_PAIRS = 3
D_OUT = 6
D_DRAM = 10                            # main DRAM slab planes (6c..6c+9)
D_SB = 9                               # plane slices per SBUF copy
P2_BASE = 48
P2_LINES = 13
P2_IN_LINES = P2_LINES + 4             # 17
P2_DRAM = 6
P2_SB = 5
CHUNKS = [(0, 9), (9, 9), (18, 9), (27, 9), (36, 9), (45, 7)]
CHUNKS2 = [(0, 7), (7, 6)]

SX = np.float32(16.0)                  # x fp8 scale (both hi and lo parts)
PLANE = 2 * S * S                      # elements per plane slice (hl-pair)
HL = S * S                             # hl sub-stride

CTR = 2                                # center tap index


def _core_assign(c):
    return 6 * c, P2_BASE + 2 * (c // 4), P2_LINES * (c % 4)


# ------------------------------------------------------- host-side weights
def _lattice_consts():
    r = np.linspace(-1.0, 1.0, SIZE, dtype=np.float32)
    lat = np.stack(np.meshgrid(r, r, r, indexing="ij"), axis=-1)
    d = np.linalg.norm(lat.astype(np.float64), axis=-1).astype(np.float32)
    values = np.linspace(0.0, 1.0, SIZE, dtype=np.float32)
    step = values[1] - values[0]
    diff = (d[..., None] - values) / step

    def sus(t):
        return np.where(t > 0, np.exp(-1.0 / np.where(t > 0, t, 1.0)), 0.0).astype(
            np.float32
        )

    emb = np.float32(1.14136) * np.float32(np.e ** 2) * sus(diff + 1.0) * sus(1.0 - diff)
    n = lat / np.maximum(d, 1e-12)[..., None]
    sh0 = np.ones_like(d)
    sh1 = np.float32(3.0 ** 0.5) * n
    return emb.astype(np.float32), sh0, sh1.astype(np.float32)


def _make_kernel(weight):
    emb, sh0, sh1 = _lattice_consts()
    w = emb @ weight
    Ssp = w.shape[:3]
    blk = MUL * MUL
    w1, w2, w3, w4 = [
        w[..., i * blk : (i + 1) * blk].reshape(*Ssp, MUL, MUL) for i in range(4)
    ]
    k_ss = PW0 * w1 * sh0[..., None, None]
    k_sv = PW1 * INV_SQRT3 * np.einsum("...uw,...k->...uwk", w2, sh1)
    k_vs = PW0 * INV_SQRT3 * np.einsum("...uw,...i->...uiw", w4, sh1)
    eye3 = np.eye(3, dtype=w.dtype)
    k_vv = (
        PW1
        * INV_SQRT3
        * (w3 * sh0[..., None, None])[..., :, None, :, None]
        * eye3[None, None, None, None, :, None, :]
    )
    top = np.concatenate([k_ss, k_sv.reshape(*Ssp, MUL, 3 * MUL)], axis=-1)
    bot = np.concatenate(
        [k_vs.reshape(*Ssp, 3 * MUL, MUL), k_vv.reshape(*Ssp, 3 * MUL, 3 * MUL)],
        axis=-1,
    )
    kernel = np.concatenate([top, bot], axis=-2)  # [5,5,5,in,out]
    return np.ascontiguousarray(np.transpose(kernel, (4, 3, 0, 1, 2)))


def _fold_self_connection(K, w_sc0, w_sc1):
    inv = np.float32(1.0 / MUL ** 0.5)
    sc = np.zeros((64, 64), np.float32)
    sc[:MUL, :MUL] = w_sc0.T * inv
    for wo in range(MUL):
        for u in range(MUL):
            for k in range(3):
                sc[MUL + 3 * wo + k, MUL + 3 * u + k] += w_sc1[u, wo] * inv
    K = K.copy()
    K[:, :, CROP, CROP, CROP] += sc
    return K


# --------------------------------------------------- matmul plan (static)
# A quad (a, dy, dx) covers slots (r, m) -> tap dz = a + r - m for output
# plane z+m, reading input planes (a, a+1) relative to the pair base.
def _col_kind(dy, dx):
    e = (dy in (0, 4)) + (dx in (0, 4))
    return ("corner", "edge", "interior")[2 - e]


def _build_plan():
    """Returns (mms, n_mm). Each mm: dict(kind='hl'|'xh', quads=[(a,dy,dx,src)],
    delta=None|int).  src: 'W8' dirty, 'WL' center correction, 'C8' center W8.
    Quads listed islot order; for 'xh' mms delta = elem offset islot0->islot1."""
    interior = [(dy, dx) for dy in (1, 2, 3) for dx in (1, 2, 3)]
    edges = [(dy, dx) for dy in range(5) for dx in range(5)
             if _col_kind(dy, dx) == "edge"]
    mms = []
    # (Q0,Q2) per interior column - planes a=0..3
    for dy, dx in interior:
        mms.append(dict(kind="xh", quads=[(0, dy, dx, "W8"), (2, dy, dx, "W8")]))
    # center tap hl-matmul (full-precision x, W8 weights, claims only dz=2)
    mms.append(dict(kind="hl", quads=[(2, 2, 2, "C8"), (2, 2, 2, "C8")]))
    # (Q1,Q3) per edge column - planes a=1..4
    for dy, dx in edges:
        mms.append(dict(kind="xh", quads=[(1, dy, dx, "W8"), (3, dy, dx, "W8")]))
    # center W_lo correction quad paired with center column's Q4
    mms.append(dict(kind="xh", quads=[(2, 2, 2, "WL"), (4, 2, 2, "W8")]))
    # leftover interior Q4 quads paired (even elem deltas only)
    for (c1, c2) in [((1, 1), (1, 3)), ((2, 1), (2, 3)), ((3, 1), (3, 3)),
                     ((1, 2), (3, 2))]:
        mms.append(dict(kind="xh", quads=[(4, c1[0], c1[1], "W8"),
                                          (4, c2[0], c2[1], "W8")]))
    for mm in mms:
        if mm["kind"] == "xh":
            (a1, dy1, dx1, _), (a2, dy2, dx2, _) = mm["quads"]
            # islot offset in (plane-slices, lines, cols); elem delta depends
            # on the tile's plane pitch (main vs partial slab geometry)
            mm["doff"] = (a2 - a1, dy2 - dy1, dx2 - dx1)
            d = (a2 - a1) * PLANE + (dy2 - dy1) * S + (dx2 - dx1)
            assert d > 0 and d % 2 == 0, mm
    return mms


MMS = _build_plan()
NMM = len(MMS)  # 27


def _pack_weights(K):
    """K [64,64,5,5,5] fp32 -> (wk fp8 [128, NMM, 2, 128], descale fp32)."""
    amax = np.abs(K).max()
    sw = np.float32(2.0 ** np.floor(np.log2(224.0 / max(amax, 1e-30))))
    K8 = (K * sw).astype(ml_dtypes.float8_e4m3)
    R = K * sw - np.asarray(K8, np.float32)          # residual, already x sw
    KL8 = R.astype(ml_dtypes.float8_e4m3)

    nz = (K ** 2).sum(axis=(0, 1)) > 0               # [5,5,5] live taps

    def needed(dy, dx, src):
        """set of (dz, m) cells this pass must produce at column (dy,dx)."""
        if src in ("C8", "WL"):
            return {(CTR, 0), (CTR, 1)} if (dy, dx) == (2, 2) else set()
        cells = {(dz, m) for dz in range(5) if nz[dz, dy, dx] for m in (0, 1)}
        if (dy, dx) == (2, 2):
            cells -= {(CTR, 0), (CTR, 1)}
        return cells

    wk = np.zeros((128, NMM, 2, 128), ml_dtypes.float8_e4m3)
    claimed = {"W8": set(), "WL": set(), "C8": set()}
    for g, mm in enumerate(MMS):
        if mm["kind"] == "hl":
            # both islots multiply (x_hi, x_lo) with IDENTICAL weights
            a, dy, dx, src = mm["quads"][0]
            for r in range(2):
                for m in range(2):
                    dz = a + r - m
                    if (dz, m) in needed(dy, dx, src):
                        claimed["C8"].add((dz, dy, dx, m))
                        for i in range(2):
                            wk[64 * r : 64 * r + 64, g, i,
                               64 * m : 64 * m + 64] = K8[:, :, dz, dy, dx].T
            continue
        for i, (a, dy, dx, src) in enumerate(mm["quads"]):
            W = {"W8": K8, "WL": KL8}[src]
            need = needed(dy, dx, src)
            for r in range(2):
                for m in range(2):
                    dz = a + r - m
                    cell = (dz, dy, dx, m)
                    if (dz, m) in need and cell not in claimed[src]:
                        claimed[src].add(cell)
                        wk[64 * r : 64 * r + 64, g, i, 64 * m : 64 * m + 64] = \
                            W[:, :, dz, dy, dx].T
    # verify full coverage
    n_dirty = sum(len(needed(dy, dx, "W8")) for dy in range(5) for dx in range(5))
    assert len(claimed["W8"]) == n_dirty, (len(claimed["W8"]), n_dirty)
    assert len(claimed["C8"]) == 2 and len(claimed["WL"]) == 2
    descale = np.float32(1.0 / (float(sw) * float(SX)))
    return np.ascontiguousarray(wk), descale


def _quant_x(x):
    """x [64,Z,Y,X] fp32 -> (xh, xl) fp8 at scale SX."""
    xs = x * SX
    xh = xs.astype(ml_dtypes.float8_e4m3)
    xl = (xs - np.asarray(xh, np.float32)).astype(ml_dtypes.float8_e4m3)
    return xh, xl


def _pack_x(x):
    """x [1,64,56,56,56] -> per-core (main [64,10,2,56,56], partial
    [64,6,2,17,56]) fp8 slabs with hl interleaved per plane."""
    xh, xl = _quant_x(np.ascontiguousarray(x[0]))
    slabs = []
    for c in range(N_CORES):
        z0, p2, y2 = _core_assign(c)
        xa = np.stack([xh[:, z0 : z0 + D_DRAM], xl[:, z0 : z0 + D_DRAM]], axis=2)
        xp = np.stack(
            [xh[:, p2 : p2 + P2_DRAM, y2 : y2 + P2_IN_LINES],
             xl[:, p2 : p2 + P2_DRAM, y2 : y2 + P2_IN_LINES]], axis=2)
        slabs.append((np.ascontiguousarray(xa), np.ascontiguousarray(xp)))
    return slabs


# ------------------------------------------------------- device program
def build_nc(descale):
    fp8 = mybir.dt.float8e4
    fp16 = mybir.dt.float16
    fp32 = mybir.dt.float32
    DR = mybir.MatmulPerfMode.DoubleRow
    nc = bacc.Bacc("TRN2", target_bir_lowering=False, debug=False,
                   num_devices=N_CORES)
    x_d = nc.dram_tensor("x", [64, D_DRAM, 2, S, S], fp8,
                         kind="ExternalInput").ap()
    x2_d = nc.dram_tensor("x2", [64, P2_DRAM, 2, P2_IN_LINES, S], fp8,
                          kind="ExternalInput").ap()
    w_d = nc.dram_tensor("w", [128, NMM, 2, 128], fp8, kind="ExternalInput").ap()
    o_d = nc.dram_tensor("out", [2 * N_PAIRS, 64, SO, SO], fp16,
                         kind="ExternalOutput").ap()
    o2_d = nc.dram_tensor("out2", [2, 64, P2_LINES, SO], fp16,
                          kind="ExternalOutput").ap()

    with tile.TileContext(nc) as tc:
        with (
            tc.tile_pool(name="const", bufs=1) as cpool,
            tc.tile_pool(name="outp", bufs=3) as opool,
            tc.tile_pool(name="psum", bufs=8, space="PSUM") as ppool,
        ):
            xt = cpool.tile([128, D_SB, 2, S, S], fp8)
            xt2 = cpool.tile([128, P2_SB, 2, P2_IN_LINES, S], fp8)
            wt = cpool.tile([128, NMM, 2, 128], fp8)

            # ---- DMAs: partial slab first (small, unblocks first chunks),
            # then weight slices interleaved with main planes in use-order.
            # xh planes: all slices lo+hi; xl only slice CTR per pair base.
            for j in range(P2_SB):
                nc.sync.dma_start(xt2[:64, j, 0], x2_d[:, j, 0])
                nc.sync.dma_start(xt2[64:, j, 0], x2_d[:, j + 1, 0])
            nc.sync.dma_start(xt2[:64, CTR, 1], x2_d[:, CTR, 1])
            nc.sync.dma_start(xt2[64:, CTR, 1], x2_d[:, CTR + 1, 1])

            wops = [
                lambda i=i: nc.sync.dma_start(wt[:, 3 * i : 3 * (i + 1)],
                                              w_d[:, 3 * i : 3 * (i + 1)])
                for i in range(NMM // 3)
            ]
            xops = []
            for j in range(D_SB):
                xops.append(lambda j=j: nc.sync.dma_start(xt[:64, j, 0],
                                                          x_d[:, j, 0]))
                xops.append(lambda j=j: nc.sync.dma_start(xt[64:, j, 0],
                                                          x_d[:, j + 1, 0]))
                if j in (2, 4, 6):
                    xops.append(lambda j=j: nc.sync.dma_start(xt[:64, j, 1],
                                                              x_d[:, j, 1]))
                    xops.append(lambda j=j: nc.sync.dma_start(xt[64:, j, 1],
                                                              x_d[:, j + 1, 1]))
            while wops or xops:
                if wops:
                    wops.pop(0)()
                if xops:
                    xops.pop(0)()

            def rhs_for(src, zb, mm, ys, L):
                if mm["kind"] == "hl":
                    a, dy, dx, _ = mm["quads"][0]
                    return src[:, zb + a, :, ys + dy : ys + dy + L,
                               dx : dx + SO]
                a1, dy1, dx1, _ = mm["quads"][0]
                base = src[:, zb + a1, 0, ys + dy1 : ys + dy1 + L,
                           dx1 : dx1 + SO]
                da, ddy, ddx = mm["doff"]
                plane_pitch = 2 * src.shape[3] * src.shape[4]
                delta = da * plane_pitch + ddy * S + ddx
                ap_list = [list(p) for p in base.ap]
                ap_list = [ap_list[0], [delta, 2]] + ap_list[1:]
                return AP(base.tensor, base.offset, ap_list)

            def do_chunk(src, zb, ys, L, dst, zo):
                ps = ppool.tile([128, 9, SO], fp32)
                for g, mm in enumerate(MMS):
                    nc.tensor.matmul(ps[:, :L, :], wt[:, g],
                                     rhs_for(src, zb, mm, ys, L),
                                     start=(g == 0), stop=(g == NMM - 1),
                                     perf_mode=DR)
                ot = opool.tile([128, 9, SO], fp16)
                nc.vector.tensor_scalar_mul(ot[:, :L], ps[:, :L], float(descale))
                dst2 = dst[zo : zo + 2, :, ys : ys + L, :].rearrange(
                    "z c l w -> (z c) l w"
                )
                nc.sync.dma_start(dst2, ot[:, :L])

            for ys, L in CHUNKS2:
                do_chunk(xt2, 0, ys, L, o2_d, 0)
            for p in range(N_PAIRS):
                for ys, L in CHUNKS:
                    do_chunk(xt, 2 * p, ys, L, o_d, 2 * p)
    nc.compile()
    return nc


# ------------------------------------------------------------ entry point
LAST_RESULTS = None
LAST_NC = None
LAST_INMAPS = None


def kernel(x, weight, w_sc0, w_sc1):
    global LAST_RESULTS, LAST_NC, LAST_INMAPS
    x = np.asarray(x, dtype=np.float32)
    K = _fold_self_connection(
        _make_kernel(np.asarray(weight, dtype=np.float32)),
        np.asarray(w_sc0, dtype=np.float32),
        np.asarray(w_sc1, dtype=np.float32),
    )
    wk, descale = _pack_weights(K)
    slabs = _pack_x(x)

    nc = build_nc(descale)
    in_maps = [{"x": slabs[c][0], "x2": slabs[c][1], "w": wk}
               for c in range(N_CORES)]
    res = run_bass_kernel_spmd(nc, in_maps, core_ids=list(range(N_CORES)))
    LAST_RESULTS, LAST_NC, LAST_INMAPS = res, nc, in_maps

    full = np.zeros((1, 64, SO, SO, SO), np.float32)
    for c in range(N_CORES):
        z0, p2, y2 = _core_assign(c)
        full[0, :, z0 : z0 + D_OUT] = np.asarray(
            res.results[c]["out"], np.float32).transpose(1, 0, 2, 3)
        full[0, :, p2 : p2 + 2, y2 : y2 + P2_LINES, :] = np.asarray(
            res.results[c]["out2"], np.float32).transpose(1, 0, 2, 3)
    return full
